# revision 19
# baseline (speedup 1.0000x reference)
"""Trainium2 Bass kernel for batched FK chain with tanh-MLP joint correction.

Math: per batch row,
    corr = tanh MLP_{7-15-15-7}(joints);  th = joints + off + corr
    M_j = DH(alpha_j, a_j, d_j, th_j);    out = (M_0 @ ... @ M_6)[:3, 3]
Factorization: M_j = A_j @ Rz(th_j) with A_j constant, and col 3 of M_6 is
constant, so the chain is 6 steps of (z-rotation + constant affine) on a
3-vector.  th_6 is never used, so the MLP's last layer only needs 6 of 7
output features.

Distribution: pure data parallel, batch/8 = 32768 rows per NeuronCore.

Per-core pipeline (pieces of the batch, pipelined against each other):
  - host packs fp16 feature-major images [128, 2048]: partition q = 64h+8k+g
    (16 batch groups x features, gap rows), free n = 128c+p, batch row
    b = 256p + 16c + 8h + g; x16 carries x_off = joints+offset (MLP input,
    bias b1' = b1 - W1 @ off absorbs the offset), xr16 carries x_red =
    range-reduced x_off (angle path, k=0..5 only)
  - 3 MLP layers as fp16 block-pattern matmuls on PE, tanh on ACT with
    per-partition bias; L3 weight block has zero-padded columns so all 128
    corr partitions are defined
  - PE transpose-mode matmuls accumulate xr16.T + corr.T (= theta.T) in PSUM
  - sin/cos via ACT Sin on half-angles (s2 = sin(th/2), s4 = sin(th/4);
    cos th = 1-2*s2^2, sin th = (2*s2)*(1-2*s4^2)) -- ACT Sin is only valid
    on [-pi, pi]; tanh+sin share one table set (silu_and_others, forced via
    a doctored table map so only one ACT table load happens)
  - chain of 6 (z-rot + const affine) steps as fp16 plane ops on DVE/GPSIMD,
    fk-derived scalars baked as immediates (program recompiled if non-joints
    inputs change; cached otherwise); [ct|st] plane pairs let one wide
    tensor_tensor produce both x*ct and y*st
  - piece i's angle/chain work overlaps piece i+1's MLP; fp16 outputs DMA
    out per piece and are upcast on host
"""

import os
import numpy as np

import concourse.bass as bass
import concourse.tile as tile
from concourse import bacc, mybir
from concourse import bass_utils

N_CORES = 8
B = 262144
BC = B // N_CORES            # 32768 rows per core
NCH = 16                     # 128-col chunks per core

F16 = mybir.dt.float16
F32 = mybir.dt.float32
AF = mybir.ActivationFunctionType
OP = mybir.AluOpType

CFG = {
    "pieces": [4, 4, 4, 4],
    "groups": [2, 2],        # pieces per chain-group (chain runs once per group)
    "warm_mms": 3,
    # engine per op: 'v' vector/DVE, 'p' gpsimd/Pool, 'a' scalar/ACT (q ops only)
    "eng": {"sq": "v", "ts": "v", "stt": "v",
            "ta": "v", "xn": "v", "t3": "p", "t4": "p", "yr": "v",
            "q1": "p", "q2": "p", "yn": "v", "zn": "v",
            "u1": "v", "x5": "v", "u3": "v", "y5": "v", "u5": "p", "z5": "v"},
    # overrides for the last piece (the exposed tail)
    "eng_tail": {"q1": "a", "q2": "a", "t3": "v", "t4": "p", "u5": "v"},
}

# constants blob column map (fp32 section)
C_BIAS1, C_BIAS2, C_BIAS3, C_PAD = 0, 1, 2, 3
C_QB1, C_QB2 = 4, 9          # -d*sin(alpha), cos(alpha)*d for j=0..4
NCONST = 16
BLOB_B = 304 * 2             # fp16 weight blocks: L1 120 | L2 120 | L3 64 cols
CONST_B = NCONST * 4
ID_B = 128 * 4
CBYTES = BLOB_B + CONST_B + ID_B


def _build_host_data(inputs):
    joints = np.asarray(inputs["joints"], np.float32)
    fk = np.asarray(inputs["fk_params"], np.float32)
    W1 = np.asarray(inputs["W1"], np.float32)
    b1 = np.asarray(inputs["b1"], np.float32)
    W2 = np.asarray(inputs["W2"], np.float32)
    b2 = np.asarray(inputs["b2"], np.float32)
    W3 = np.asarray(inputs["W3"], np.float32)
    b3 = np.asarray(inputs["b3"], np.float32)

    off = fk[:, 3]
    b1p = b1 - W1 @ off
    x_off = joints + off[None, :]          # [B, 7] fp32
    # exact host range-reduction for the angle path (Sin on ACT needs [-pi, pi];
    # device uses half-angle identities so th = x_red + corr stays in range)
    x_red = (np.remainder(x_off + np.pi, 2 * np.pi) - np.pi).astype(np.float32)

    # --- per-core feature-major images ---
    # batch row b = 256*p + 16*c + 8*h + g; partition q = 64*h + 8*k + g
    # img[q, 128*c + p] = src[b, k]
    def mkimg(src, nk):
        out = []
        for core in range(N_CORES):
            jc = src[core * BC:(core + 1) * BC, :nk]        # [32768, nk]
            arr = jc.reshape(128, 16, 2, 8, nk)             # [p, c, h, g, k]
            arr = arr.transpose(2, 4, 3, 1, 0)              # [h, k, g, c, p]
            img = np.zeros((2, 8, 8, 16, 128), np.float32)
            img[:, :nk] = arr
            out.append(np.ascontiguousarray(
                img.reshape(128, 2048)).astype(np.float16))
        return out
    imgs_x = mkimg(x_off, 7)
    # fp32 image for the angle path (transposed+accumulated with corr in psum)
    imgs_r = []
    for core in range(N_CORES):
        jc = x_red[core * BC:(core + 1) * BC, :6]
        arr = jc.reshape(128, 16, 2, 8, 6).transpose(2, 4, 3, 1, 0)
        img = np.zeros((2, 8, 8, 16, 128), np.float32)
        img[:, :6] = arr
        imgs_r.append(np.ascontiguousarray(img.reshape(128, 2048)))

    # --- block-pattern weights (fp16), packed into one [128, 304] blob ---
    blob = np.zeros((128, 304), np.float16)
    # L1: lhsT1[64h+8k+g, 15g+j] = W1[j, k]  (cols 0:120)
    for h in (0, 1):
        for k in range(7):
            for g in range(8):
                blob[64 * h + 8 * k + g, 15 * g:15 * g + 15] = W1[:, k]
    # L2: lhsT2[15g+i, 15g+j] = W2[j, i]  (cols 120:240)
    for g in range(8):
        blob[15 * g:15 * g + 15, 120 + 15 * g:120 + 15 * g + 15] = W2.T
    # L3: lhsT3[15g+i, 8k+g] = W3[k, i] for k<6 (cols 240:288); cols 288:304
    # stay zero so psum rows 48:64 / 112:128 are defined (tanh(0)=0)
    for g in range(8):
        for k in range(6):
            blob[15 * g:15 * g + 15, 240 + 8 * k + g] = W3[k, :]

    # --- constants blob [128, NCONST] fp32 ---
    consts = np.zeros((128, NCONST), np.float32)
    for g in range(8):
        for j in range(15):
            consts[15 * g + j, C_BIAS1] = b1p[j]
            consts[15 * g + j, C_BIAS2] = b2[j]
    for h in (0, 1):
        for k in range(6):
            for g in range(8):
                consts[64 * h + 8 * k + g, C_BIAS3] = b3[k]
    alpha, dd = fk[:, 0], fk[:, 2]
    for j in range(5):
        consts[:, C_QB1 + j] = -dd[j] * np.sin(alpha[j])
        consts[:, C_QB2 + j] = np.cos(alpha[j]) * dd[j]

    id32 = np.eye(128, dtype=np.float32)
    cblob = np.concatenate([
        blob.view(np.uint8).reshape(128, BLOB_B),
        consts.view(np.uint8).reshape(128, CONST_B),
        np.ascontiguousarray(id32).view(np.uint8).reshape(128, ID_B),
    ], axis=1)
    return imgs_x, imgs_r, np.ascontiguousarray(cblob)


def _emit_program(nc, sc, reps=1, loop_n=0):
    dx16 = nc.dram_tensor("x16", [128, 2048], F16, kind="ExternalInput")
    dxr32 = nc.dram_tensor("xr32", [128, 2048], F32, kind="ExternalInput")
    dcblob = nc.dram_tensor("cblob", [128, CBYTES], mybir.dt.uint8,
                            kind="ExternalInput")
    dout = nc.dram_tensor("out", [128, 768], F16, kind="ExternalOutput")

    pieces = list(CFG["pieces"])
    assert sum(pieces) == NCH
    MAXC = max(pieces)
    MN = 128 * MAXC

    from contextlib import ExitStack, nullcontext
    with tile.TileContext(nc) as tc, ExitStack() as ctx:
        cp = ctx.enter_context(tc.tile_pool(name="persist", bufs=1))
        pp = ctx.enter_context(tc.tile_pool(name="piecep", bufs=2))
        chp = ctx.enter_context(tc.tile_pool(name="chain", bufs=4))
        mlp_ps = ctx.enter_context(tc.tile_pool(name="mlpps", bufs=2, space="PSUM"))
        l3_ps = ctx.enter_context(tc.tile_pool(name="l3ps", bufs=1, space="PSUM"))
        tp_ps = ctx.enter_context(tc.tile_pool(name="tpps", bufs=1, space="PSUM"))

        cblob = cp.tile([128, CBYTES], mybir.dt.uint8, tag="cblob")
        lhs = cblob[:, 0:BLOB_B].bitcast(F16)
        consts = cblob[:, BLOB_B:BLOB_B + CONST_B].bitcast(F32)
        id32 = cblob[:, BLOB_B + CONST_B:CBYTES].bitcast(F32)

        # hoist the ACT table load under the input DMAs
        warm = cp.tile([128, 1], F32, tag="warm")
        nc.vector.memset(warm[:], 0.0)
        nc.scalar.activation(warm[:], warm[:], AF.Tanh, bias=0.0)

        def cv(col, parts=128):
            return consts[0:parts, col:col + 1]

        def eng(op, tail):
            e = CFG["eng_tail"].get(op) if tail else None
            e = e or CFG["eng"][op]
            return {"v": nc.vector, "p": nc.gpsimd, "a": nc.scalar}[e]

        # PE warm-up: dummy matmuls on a memset tile so the PE clock ramps
        # while the input DMAs are in flight.
        wm16 = cp.tile([128, 128], F16, tag="wm16")
        nc.vector.memset(wm16[:], 0.0)
        wmps = mlp_ps.tile([128, MN], F32, tag="mlpps", name="wmps")
        for _w in range(CFG.get("warm_mms", 3)):
            nc.tensor.matmul(wmps[:, 0:128], wm16[0:64, 0:128],
                             wm16[0:64, 0:128], start=True, stop=True)

        def l1_mms(x16t, c0, C):
            """L1 matmuls for a piece; returns psum tiles for the tanh."""
            npc = 128 * C
            pss = []
            for h in (0, 1):
                ps = mlp_ps.tile([128, MN], F32, tag="mlpps", name="l1ps")
                for so in range(0, npc, 512):
                    sw = min(512, npc - so)
                    nc.tensor.matmul(
                        ps[0:120, so:so + sw],
                        lhs[64 * h:64 * h + 64, 0:120],
                        x16t[64 * h:64 * h + 64, 128 * c0 + so:128 * c0 + so + sw],
                        start=True, stop=True, tile_position=(64 * h, 0))
                pss.append(ps)
            return pss

        def mlp_rest(l1ps, C):
            """L1 tanh, L2, L3 for a piece; returns corr [128, npc] f16."""
            npc = 128 * C
            h1sb = pp.tile([128, 2 * MN], F16, tag="h1", name="h1")
            h2sb = pp.tile([128, 2 * MN], F16, tag="h2", name="h2")
            if len(l1ps) == 1:      # both h halves in one psum tile
                nc.scalar.activation(
                    h1sb[0:120, 0:2 * npc],
                    l1ps[0][0:120, 0:2 * npc], AF.Tanh, bias=cv(C_BIAS1, 120))
                ps = mlp_ps.tile([128, PSW], F32, tag="mlpps", name="l2ps")
                for h in (0, 1):
                    for so in range(0, npc, 512):
                        sw = min(512, npc - so)
                        nc.tensor.matmul(
                            ps[0:120, npc * h + so:npc * h + so + sw],
                            lhs[0:120, 120:240],
                            h1sb[0:120, npc * h + so:npc * h + so + sw],
                            start=True, stop=True)
                nc.scalar.activation(
                    h2sb[0:120, 0:2 * npc],
                    ps[0:120, 0:2 * npc], AF.Tanh, bias=cv(C_BIAS2, 120))
            else:
                for h in (0, 1):
                    nc.scalar.activation(
                        h1sb[0:120, npc * h:npc * h + npc],
                        l1ps[h][0:120, 0:npc], AF.Tanh, bias=cv(C_BIAS1, 120))
                for h in (0, 1):
                    ps = mlp_ps.tile([128, PSW], F32, tag="mlpps", name="l2ps")
                    for so in range(0, npc, 512):
                        sw = min(512, npc - so)
                        nc.tensor.matmul(
                            ps[0:120, so:so + sw],
                            lhs[0:120, 120:240],
                            h1sb[0:120, npc * h + so:npc * h + so + sw],
                            start=True, stop=True)
                    nc.scalar.activation(
                        h2sb[0:120, npc * h:npc * h + npc],
                        ps[0:120, 0:npc], AF.Tanh, bias=cv(C_BIAS2, 120))
            ps3 = l3_ps.tile([128, MN], F32, tag="l3ps", name="l3ps")
            for h in (0, 1):
                for so in range(0, npc, 512):
                    sw = min(512, npc - so)
                    nc.tensor.matmul(
                        ps3[64 * h:64 * h + 64, so:so + sw],
                        lhs[0:120, 240:304],
                        h2sb[0:120, npc * h + so:npc * h + so + sw],
                        start=True, stop=True, tile_position=(0, 64 * h))
            corr = pp.tile([128, MN], F32, tag="corr", name="corr")
            nc.scalar.activation(corr[:, 0:npc], ps3[:, 0:npc],
                                 AF.Tanh, bias=cv(C_BIAS3))
            return corr

        def angle_part(xr32t, corr, CSg, gc0, GW, c0, C, tail):
            """Transpose + sin + combine for one piece, writing planes into its
            chain-group's CS tile ([t4 | ct | st] plane layout, GW chunks)."""
            npc = 128 * C
            tps = tp_ps.tile([128, MN], F32, tag="tpps", name="tps")
            for ci in range(C):
                nc.tensor.matmul(tps[:, 128 * ci:128 * ci + 128],
                                 xr32t[:, 128 * (c0 + ci):128 * (c0 + ci) + 128],
                                 id32[:], is_transpose=True, start=True, stop=False)
                nc.tensor.matmul(tps[:, 128 * ci:128 * ci + 128],
                                 corr[:, 128 * ci:128 * ci + 128],
                                 id32[:], is_transpose=True, start=False, stop=True)
            S24 = pp.tile([128, 2 * 96 * MAXC], F16, tag="S24", name="S24")
            in_v = tps[:, 0:npc].rearrange(
                "p (c h k g) -> p c h k g", c=C, h=2, k=8, g=8)[:, :, :, 0:6, :]
            s4v = S24[:, 0:96 * C].rearrange(
                "p (k c h g) -> p c h k g", k=6, c=C, h=2, g=8)
            s2v = S24[:, 96 * C:192 * C].rearrange(
                "p (k c h g) -> p c h k g", k=6, c=C, h=2, g=8)
            nc.scalar.activation(s2v, in_v, AF.Sin, bias=0.0, scale=0.5)
            nc.scalar.activation(s4v, in_v, AF.Sin, bias=0.0, scale=0.25)
            SQ = pp.tile([128, 2 * 96 * MAXC], F16, tag="SQ", name="SQ")
            eng("sq", tail).tensor_tensor(
                SQ[:, 0:192 * C], S24[:, 0:192 * C], S24[:, 0:192 * C], OP.mult)
            # [sq4 | sq2] -> [t4 | ct] planes of the group CS
            pc0 = c0 - gc0
            sq_v = SQ[:, 0:192 * C].rearrange(
                "p (x k c h g) -> p x k c h g", x=2, k=6, c=C, h=2, g=8)
            cs_v = CSg[:, 0:288 * GW].rearrange(
                "p (x k c h g) -> p x k c h g", x=3, k=6, c=GW, h=2, g=8)
            eng("ts", tail).tensor_scalar(
                cs_v[:, 0:2, :, pc0:pc0 + C], sq_v, -2.0, 1.0, OP.mult, OP.add)
            s2_v = S24[:, 96 * C:192 * C].rearrange(
                "p (k c h g) -> p k c h g", k=6, c=C, h=2, g=8)
            eng("stt", tail).scalar_tensor_tensor(
                cs_v[:, 2, :, pc0:pc0 + C], s2_v, 2.0,
                cs_v[:, 0, :, pc0:pc0 + C], OP.mult, OP.mult)

        def chain_part(CSg, pack, gc0, GW, tail):
            PL = 16 * GW
            m0 = 16 * gc0

            def ctj(j):
                return CSg[:, 96 * GW + 16 * GW * j:96 * GW + 16 * GW * j + PL]

            def stj(j):
                return CSg[:, 192 * GW + 16 * GW * j:192 * GW + 16 * GW * j + PL]

            CSv = CSg[:, 0:288 * GW].rearrange("p (x k m) -> p k x m",
                                               x=3, k=6, m=PL)

            def ch(tag, w=1):
                return chp.tile([128, w * 16 * NCH], F16, tag=tag, name=tag)

            def pv(t):
                return t[:, 0:2 * PL].rearrange("p (o x m) -> p o x m",
                                                o=1, x=2, m=PL)

            P = ch("P", 2)
            z = ch("z")
            u1 = ch("u1")
            eng("u1", tail).tensor_scalar(u1[:, 0:PL], ctj(5), sc["s5u1m"],
                                          sc["s5u1a"], OP.mult, OP.add)
            eng("x5", tail).scalar_tensor_tensor(P[:, 0:PL], stj(5), sc["s5xm"],
                                                 u1[:, 0:PL], OP.mult, OP.add)
            u3 = ch("u3")
            eng("u3", tail).tensor_scalar(u3[:, 0:PL], stj(5), sc["s5u3m"],
                                          sc["s5u3a"], OP.mult, OP.add)
            eng("y5", tail).scalar_tensor_tensor(P[:, PL:2 * PL], ctj(5),
                                                 sc["s5ym"], u3[:, 0:PL],
                                                 OP.mult, OP.add)
            u5 = ch("u5")
            eng("u5", tail).tensor_scalar(u5[:, 0:PL], stj(5), sc["s5u5m"],
                                          sc["s5u5a"], OP.mult, OP.add)
            eng("z5", tail).scalar_tensor_tensor(z[:, 0:PL], ctj(5), sc["s5zm"],
                                                 u5[:, 0:PL], OP.mult, OP.add)

            for j in (4, 3, 2, 1, 0):
                last = j == 0
                a_j, ca_j, sa_j = sc[f"a{j}"], sc[f"ca{j}"], sc[f"sa{j}"]
                dsa_j, cad_j = sc[f"dsa{j}"], sc[f"cad{j}"]
                TA = ch("TA", 2)
                pair = CSv[:, j:j + 1, 1:3, :]
                eng("ta", tail).tensor_tensor(pv(TA), pv(P), pair, OP.mult)
                Pn = pack if last else ch("P", 2)
                xn = Pn[:, m0:m0 + PL] if last else Pn[:, 0:PL]
                eng("xn", tail).scalar_tensor_tensor(
                    xn, TA[:, 0:PL], a_j, TA[:, PL:2 * PL], OP.add, OP.subtract)
                t3 = ch("t3")
                eng("t3", tail).tensor_tensor(t3[:, 0:PL], P[:, 0:PL], stj(j),
                                              OP.mult)
                t4 = ch("t4")
                eng("t4", tail).tensor_tensor(t4[:, 0:PL], P[:, PL:2 * PL],
                                              ctj(j), OP.mult)
                yr = ch("yr")
                eng("yr", tail).tensor_tensor(yr[:, 0:PL], t3[:, 0:PL],
                                              t4[:, 0:PL], OP.add)
                q1 = ch("q1")
                e = eng("q1", tail)
                if e is nc.scalar:
                    e.activation(q1[:, 0:PL], z[:, 0:PL], AF.Identity,
                                 bias=cv(C_QB1 + j), scale=-sa_j)
                else:
                    e.tensor_scalar(q1[:, 0:PL], z[:, 0:PL], -sa_j, -dsa_j,
                                    OP.mult, OP.add)
                q2 = ch("q2")
                e = eng("q2", tail)
                if e is nc.scalar:
                    e.activation(q2[:, 0:PL], z[:, 0:PL], AF.Identity,
                                 bias=cv(C_QB2 + j), scale=ca_j)
                else:
                    e.tensor_scalar(q2[:, 0:PL], z[:, 0:PL], ca_j, cad_j,
                                    OP.mult, OP.add)
                yn = Pn[:, 256 + m0:256 + m0 + PL] if last else Pn[:, PL:2 * PL]
                eng("yn", tail).scalar_tensor_tensor(
                    yn, yr[:, 0:PL], ca_j, q1[:, 0:PL], OP.mult, OP.add)
                zn = pack[:, 512 + m0:512 + m0 + PL] if last else ch("z")
                znv = zn if last else zn[:, 0:PL]
                eng("zn", tail).scalar_tensor_tensor(
                    znv, yr[:, 0:PL], sa_j, q2[:, 0:PL], OP.mult, OP.add)
                P, z = Pn, zn

        loop_ctx = tc.For_i(0, loop_n, 1) if loop_n else nullcontext()
        first = True
        with loop_ctx:
          for _rep in range(reps):
            x16t = cp.tile([128, 2048], F16, tag="x16", name="x16")
            xr32t = cp.tile([128, 2048], F32, tag="xr32", name="xr32")
            pack = cp.tile([128, 768], F16, tag="pack", name="pack")
            c0s = [sum(pieces[:i]) for i in range(len(pieces))]
            n1 = 128 * pieces[0]
            # piece-1 inputs first (they gate the whole pipeline), then the rest
            nc.sync.dma_start(x16t[:, 0:n1], dx16.ap()[:, 0:n1])
            if first:
                nc.sync.dma_start(cblob[:], dcblob.ap())
                first = False
            nc.sync.dma_start(xr32t[:, 0:n1], dxr32.ap()[:, 0:n1])
            if n1 < 2048:
                nc.sync.dma_start(x16t[:, n1:2048], dx16.ap()[:, n1:2048])
                nc.sync.dma_start(xr32t[:, n1:2048], dxr32.ap()[:, n1:2048])
            groups = list(CFG["groups"])
            assert sum(groups) == len(pieces)
            group_idxs = []
            s = 0
            for g in groups:
                group_idxs.append(list(range(s, s + g)))
                s += g
            gl = {}          # piece idx -> (group tile, gc0, GW, is_last_piece)
            for gi, idxs in enumerate(group_idxs):
                GW = sum(pieces[i] for i in idxs)
                gc0 = c0s[idxs[0]]
                CSg = cp.tile([128, 288 * GW], F16, tag=f"CS{gi}", name=f"CS{gi}")
                for i in idxs:
                    gl[i] = (CSg, gc0, GW, gi, i == idxs[-1])
            l1ps = l1_mms(x16t, c0s[0], pieces[0])
            for pi, (c0, C) in enumerate(zip(c0s, pieces)):
                corr = mlp_rest(l1ps, C)
                if pi + 1 < len(pieces):
                    l1ps = l1_mms(x16t, c0s[pi + 1], pieces[pi + 1])
                CSg, gc0, GW, gi, last_in_group = gl[pi]
                tail = gi == len(group_idxs) - 1
                if CFG.get("hp_angle", True):
                    with tc.high_priority():
                        angle_part(xr32t, corr, CSg, gc0, GW, c0, C, tail)
                else:
                    angle_part(xr32t, corr, CSg, gc0, GW, c0, C, tail)
                if last_in_group:
                    if CFG.get("hp_angle", True):
                        with tc.high_priority():
                            chain_part(CSg, pack, gc0, GW, tail)
                    else:
                        chain_part(CSg, pack, gc0, GW, tail)
                    m0 = 16 * gc0
                    PL = 16 * GW
                    dv = dout.ap().rearrange("p (t m) -> p t m", t=3, m=256)
                    pvw = pack[:, 0:768].rearrange("p (t m) -> p t m", t=3, m=256)
                    if tail:
                        for t3i in range(3):
                            nc.sync.dma_start(dv[:, t3i:t3i + 1, m0:m0 + PL],
                                              pvw[:, t3i:t3i + 1, m0:m0 + PL])
                    else:
                        nc.sync.dma_start(dv[:, :, m0:m0 + PL],
                                          pvw[:, :, m0:m0 + PL])


_PROG_CACHE = {}


def _baked_scalars(inputs):
    fk = np.asarray(inputs["fk_params"], np.float32)
    alpha, a, d = fk[:, 0], fk[:, 1], fk[:, 2]
    ca, sa = np.cos(alpha), np.sin(alpha)
    t6 = np.array([a[6], -d[6] * sa[6], ca[6] * d[6]], np.float32)
    sc = {
        "s5u1m": a[6], "s5u1a": a[5], "s5xm": -t6[1],
        "s5u3m": ca[5] * a[6], "s5u3a": -sa[5] * t6[2] - d[5] * sa[5],
        "s5ym": ca[5] * t6[1],
        "s5u5m": sa[5] * a[6], "s5u5a": ca[5] * t6[2] + ca[5] * d[5],
        "s5zm": sa[5] * t6[1],
    }
    for j in range(5):
        sc[f"a{j}"] = a[j]
        sc[f"ca{j}"] = ca[j]
        sc[f"sa{j}"] = sa[j]
        sc[f"dsa{j}"] = d[j] * sa[j]
        sc[f"cad{j}"] = ca[j] * d[j]
    return {k: float(np.float32(v)) for k, v in sc.items()}


def _cfg_key():
    return (tuple(CFG["pieces"]), tuple(sorted(CFG["eng"].items())),
            tuple(sorted(CFG["eng_tail"].items())))


def _get_program(inputs, reps=1, loop_n=0):
    sc = _baked_scalars(inputs)
    key = (tuple(sorted(sc.items())), reps, loop_n, _cfg_key())
    if key in _PROG_CACHE:
        return _PROG_CACHE[key]
    nc = bacc.Bacc("TRN2", target_bir_lowering=False, debug=False,
                   enable_asserts=False)
    _emit_program(nc, sc, reps=reps, loop_n=loop_n)

    # Force Tanh and Sin to resolve to the one table set containing both
    # (silu_and_others), so the kernel pays a single ACT table load.
    import concourse.bacc as bacc_mod
    from concourse.hw_specs import get_activation_tables
    orig_fn = bacc_mod.get_activation_tables
    tabs = get_activation_tables(nc.m.arch)
    trig = {AF.Tanh, AF.Sin}
    doctored = {
        name: (set(funcs) if name == "silu_and_others" else set(funcs) - trig)
        for name, funcs in tabs.items()
    }
    bacc_mod.get_activation_tables = lambda arch: doctored
    try:
        nc.compile()
    finally:
        bacc_mod.get_activation_tables = orig_fn

    _PROG_CACHE[key] = nc
    return nc


LAST_RESULTS = None  # BassKernelResults of the most recent run (for test.py)


def _host_in_maps(inputs):
    imgs_x, imgs_r, cblob = _build_host_data(inputs)
    in_maps = []
    for core in range(N_CORES):
        in_maps.append({
            "x16": imgs_x[core],
            "xr32": imgs_r[core],
            "cblob": cblob,
        })
    return in_maps


def _jit_runner(nc):
    import jax
    from jax.sharding import Mesh, PartitionSpec, NamedSharding
    from jax.experimental.shard_map import shard_map
    from concourse import bass2jax
    bass2jax.install_neuronx_cc_hook()

    partition_name = nc.partition_id_tensor.name if nc.partition_id_tensor else None
    in_names, out_names, out_avals = [], [], []
    for alloc in nc.m.functions[0].allocations:
        if not isinstance(alloc, mybir.MemoryLocationSet):
            continue
        name = alloc.memorylocations[0].name
        if alloc.kind == "ExternalInput":
            if name != partition_name:
                in_names.append(name)
        elif alloc.kind == "ExternalOutput":
            out_names.append(name)
            out_avals.append(jax.core.ShapedArray(
                tuple(alloc.tensor_shape), mybir.dt.np(alloc.dtype)))
    all_in = in_names + out_names + ([partition_name] if partition_name else [])
    devices = jax.devices()[:N_CORES]
    mesh = Mesh(np.asarray(devices), ("core",))
    sh = NamedSharding(mesh, PartitionSpec("core"))

    def _body(*args):
        ops = list(args)
        if partition_name:
            ops.append(bass2jax.partition_id_tensor())
        outs = bass2jax._bass_exec_p.bind(
            *ops, out_avals=tuple(out_avals), in_names=tuple(all_in),
            out_names=tuple(out_names), lowering_input_output_aliases=(),
            sim_require_finite=True, sim_require_nnan=True, nc=nc)
        return tuple(outs)

    specs = (PartitionSpec("core"),) * (len(in_names) + len(out_names))
    ospec = (PartitionSpec("core"),) * len(out_names)
    f = jax.jit(shard_map(_body, mesh=mesh, in_specs=specs, out_specs=ospec,
                          check_rep=False))
    return f, in_names, out_avals, sh


def time_on_hw(inputs, n_lo=16, n_hi=256, iters=10):
    """Per-kernel HW time via an on-device For_i loop: slope of min wall
    between trip counts (includes ~2us loop back-edge per iteration)."""
    import time as _time
    import jax
    in_maps = _host_in_maps(inputs)
    mins = {}
    for loop_n in (n_lo, n_hi):
        nc = _get_program(inputs, loop_n=loop_n)
        f, in_names, out_avals, sh = _jit_runner(nc)
        cat = lambda n: np.concatenate(
            [np.asarray(in_maps[c][n]) for c in range(N_CORES)], axis=0)
        ci = [jax.device_put(cat(n), sh) for n in in_names]
        cz = [jax.device_put(
            np.zeros((N_CORES * a.shape[0], *a.shape[1:]), a.dtype), sh)
            for a in out_avals]
        jax.block_until_ready(f(*ci, *cz))
        best = float("inf")
        for _ in range(iters):
            t0 = _time.perf_counter()
            jax.block_until_ready(f(*ci, *cz))
            best = min(best, _time.perf_counter() - t0)
        mins[loop_n] = best
        print(f"[hw timing] loop_n={loop_n}: min wall {best*1e3:.2f} ms")
    slope_ns = (mins[n_hi] - mins[n_lo]) / (n_hi - n_lo) * 1e9
    print(f"[hw timing] -> {slope_ns:.0f} ns/kernel (incl ~2us loop overhead)")
    return slope_ns


def kernel(**inputs):
    global LAST_RESULTS
    j = np.asarray(inputs["joints"])
    assert j.shape == (B, 7), f"kernel hardcodes joints shape {(B, 7)}, got {j.shape}"
    nc = _get_program(inputs)
    in_maps = _host_in_maps(inputs)
    res = bass_utils.run_bass_kernel_spmd(nc, in_maps, core_ids=list(range(N_CORES)))
    LAST_RESULTS = res

    out = np.empty((B, 3), np.float32)
    for core in range(N_CORES):
        p = np.asarray(res.results[core]["out"]).astype(np.float32)  # [128, 768]
        # pack cols: [px(256) | py(256) | pz(256)], b_local = 256*p + m
        oc = p.reshape(128, 3, 256).transpose(0, 2, 1).reshape(BC, 3)
        out[core * BC:(core + 1) * BC] = oc
    return out


# revision 27
# speedup vs baseline: 1.2883x; 1.2883x over previous
"""Trainium2 Bass kernel for batched FK chain with tanh-MLP joint correction.

Math: per batch row,
    corr = tanh MLP_{7-15-15-7}(joints);  th = joints + off + corr
    M_j = DH(alpha_j, a_j, d_j, th_j);    out = (M_0 @ ... @ M_6)[:3, 3]
Factorization: M_j = A_j @ Rz(th_j) with A_j constant, and col 3 of M_6 is
constant, so the chain is 6 steps of (z-rotation + constant affine) on a
3-vector.  th_6 is never used, so the MLP's last layer only needs 6 of 7
output features.

Distribution: pure data parallel, batch/8 = 32768 rows per NeuronCore.

Per-core pipeline: the batch is split into MLP "pieces" (default 4x4 chunks
of 128 cols) that stream through PE+ACT, and into chain "groups" (default 2
pieces each) so the serial rotation chain runs half as many, twice-as-wide
elementwise instructions:
  - host packs feature-major images: partition q = 64h+8k+g (16 batch groups
    x features, gap rows), free n = 128c+p, batch row b = 256p+16c+8h+g;
    x16 (fp16) carries x_off = joints+offset (MLP input, bias b1' =
    b1 - W1 @ off absorbs the offset), xr32 (fp32) carries range-reduced
    x_off for the angle path (k=0..5 only -- th_6 is unused)
  - 3 MLP layers as fp16 block-pattern matmuls on PE, tanh on ACT with
    per-partition bias; the L3 weight block computes only 6 corr features
    and has zero-padded columns so all 128 corr partitions are defined
  - PE transpose-mode matmuls accumulate xr32.T + corr.T (= theta.T) in PSUM
  - sin/cos via ACT Sin on half-angles (s2 = sin(th/2), s4 = sin(th/4);
    cos th = 1-2*s2^2, sin th = (2*s2)*(1-2*s4^2)) -- ACT Sin is only valid
    on [-pi, pi]; tanh+sin share one table set (silu_and_others, forced via
    a doctored table map so only one ACT table load happens); S24 = [s4|s2]
    so ONE wide affine yields [1-2*s4^2 | cos th] contiguously and the
    combine is 3 instructions per piece writing [t4 | ct | st] group planes
  - chain of 6 (z-rot + const affine) steps as fp16 plane ops per group,
    split across DVE/GPSIMD/ACT by a tunable map (q1/q2 hoisted a step
    early; [ct|st] plane pairs let one wide tensor_tensor produce x*ct and
    y*st together); fk-derived scalars baked as immediates (program is
    recompiled if the non-joints inputs change; cached otherwise)
  - piece i's angle work overlaps piece i+1's MLP (high_priority hints);
    fp16 outputs DMA out per group and are upcast on host
"""

import os
import numpy as np

import concourse.bass as bass
import concourse.tile as tile
from concourse import bacc, mybir
from concourse import bass_utils

N_CORES = 8
B = 262144
BC = B // N_CORES            # 32768 rows per core
NCH = 16                     # 128-col chunks per core

F16 = mybir.dt.float16
F32 = mybir.dt.float32
AF = mybir.ActivationFunctionType
OP = mybir.AluOpType

CFG = {
    "pieces": [4, 4, 4, 4],
    "groups": [2, 2],        # pieces per chain-group (chain runs once per group)
    "warm_mms": 3,
    # engine per op: 'v' vector/DVE, 'p' gpsimd/Pool, 'a' scalar/ACT (q ops only)
    "eng": {"sq": "v", "ts": "v", "stt": "v",
            "ta": "v", "xn": "v", "t3": "p", "t4": "p", "yr": "v",
            "q1": "p", "q2": "p", "yn": "v", "zn": "v",
            "u1": "v", "x5": "v", "u3": "v", "y5": "v", "u5": "p", "z5": "v"},
    # overrides for the last piece (the exposed tail)
    "eng_tail": {"q1": "p", "q2": "p", "t3": "v", "t4": "v", "u5": "v"},
}

# constants blob column map (fp32 section)
C_BIAS1, C_BIAS2, C_BIAS3, C_PAD = 0, 1, 2, 3
C_QB1, C_QB2 = 4, 9          # -d*sin(alpha), cos(alpha)*d for j=0..4
NCONST = 16
BLOB_B = 304 * 2             # fp16 weight blocks: L1 120 | L2 120 | L3 64 cols
CONST_B = NCONST * 4
ID_B = 128 * 4
CBYTES = BLOB_B + CONST_B + ID_B


def _build_host_data(inputs):
    joints = np.asarray(inputs["joints"], np.float32)
    fk = np.asarray(inputs["fk_params"], np.float32)
    W1 = np.asarray(inputs["W1"], np.float32)
    b1 = np.asarray(inputs["b1"], np.float32)
    W2 = np.asarray(inputs["W2"], np.float32)
    b2 = np.asarray(inputs["b2"], np.float32)
    W3 = np.asarray(inputs["W3"], np.float32)
    b3 = np.asarray(inputs["b3"], np.float32)

    off = fk[:, 3]
    b1p = b1 - W1 @ off
    x_off = joints + off[None, :]          # [B, 7] fp32
    # exact host range-reduction for the angle path (Sin on ACT needs [-pi, pi];
    # device uses half-angle identities so th = x_red + corr stays in range)
    x_red = (np.remainder(x_off + np.pi, 2 * np.pi) - np.pi).astype(np.float32)

    # --- per-core feature-major images ---
    # batch row b = 256*p + 16*c + 8*h + g; partition q = 64*h + 8*k + g
    # img[q, 128*c + p] = src[b, k]
    def mkimg(src, nk):
        out = []
        for core in range(N_CORES):
            jc = src[core * BC:(core + 1) * BC, :nk]        # [32768, nk]
            arr = jc.reshape(128, 16, 2, 8, nk)             # [p, c, h, g, k]
            arr = arr.transpose(2, 4, 3, 1, 0)              # [h, k, g, c, p]
            img = np.zeros((2, 8, 8, 16, 128), np.float32)
            img[:, :nk] = arr
            out.append(np.ascontiguousarray(
                img.reshape(128, 2048)).astype(np.float16))
        return out
    imgs_x = mkimg(x_off, 7)
    # fp32 image for the angle path (transposed+accumulated with corr in psum)
    imgs_r = []
    for core in range(N_CORES):
        jc = x_red[core * BC:(core + 1) * BC, :6]
        arr = jc.reshape(128, 16, 2, 8, 6).transpose(2, 4, 3, 1, 0)
        img = np.zeros((2, 8, 8, 16, 128), np.float32)
        img[:, :6] = arr
        imgs_r.append(np.ascontiguousarray(img.reshape(128, 2048)))

    # --- block-pattern weights (fp16), packed into one [128, 304] blob ---
    blob = np.zeros((128, 304), np.float16)
    # L1: lhsT1[64h+8k+g, 15g+j] = W1[j, k]  (cols 0:120)
    for h in (0, 1):
        for k in range(7):
            for g in range(8):
                blob[64 * h + 8 * k + g, 15 * g:15 * g + 15] = W1[:, k]
    # L2: lhsT2[15g+i, 15g+j] = W2[j, i]  (cols 120:240)
    for g in range(8):
        blob[15 * g:15 * g + 15, 120 + 15 * g:120 + 15 * g + 15] = W2.T
    # L3: lhsT3[15g+i, 8k+g] = W3[k, i] for k<6 (cols 240:288); cols 288:304
    # stay zero so psum rows 48:64 / 112:128 are defined (tanh(0)=0)
    for g in range(8):
        for k in range(6):
            blob[15 * g:15 * g + 15, 240 + 8 * k + g] = W3[k, :]

    # --- constants blob [128, NCONST] fp32 ---
    consts = np.zeros((128, NCONST), np.float32)
    for g in range(8):
        for j in range(15):
            consts[15 * g + j, C_BIAS1] = b1p[j]
            consts[15 * g + j, C_BIAS2] = b2[j]
    for h in (0, 1):
        for k in range(6):
            for g in range(8):
                consts[64 * h + 8 * k + g, C_BIAS3] = b3[k]
    alpha, dd = fk[:, 0], fk[:, 2]
    for j in range(5):
        consts[:, C_QB1 + j] = -dd[j] * np.sin(alpha[j])
        consts[:, C_QB2 + j] = np.cos(alpha[j]) * dd[j]

    id32 = np.eye(128, dtype=np.float32)
    cblob = np.concatenate([
        blob.view(np.uint8).reshape(128, BLOB_B),
        consts.view(np.uint8).reshape(128, CONST_B),
        np.ascontiguousarray(id32).view(np.uint8).reshape(128, ID_B),
    ], axis=1)
    return imgs_x, imgs_r, np.ascontiguousarray(cblob)


def _emit_program(nc, sc, reps=1, loop_n=0):
    dx16 = nc.dram_tensor("x16", [128, 2048], F16, kind="ExternalInput")
    dxr32 = nc.dram_tensor("xr32", [128, 2048], F32, kind="ExternalInput")
    dcblob = nc.dram_tensor("cblob", [128, CBYTES], mybir.dt.uint8,
                            kind="ExternalInput")
    dout = nc.dram_tensor("out", [128, 768], F16, kind="ExternalOutput")

    pieces = list(CFG["pieces"])
    assert sum(pieces) == NCH
    MAXC = max(pieces)
    MN = 128 * MAXC
    PSW = MN if MAXC > 4 else 2 * MN     # mlp psum tile width (fp32 cols)

    from contextlib import ExitStack, nullcontext
    with tile.TileContext(nc) as tc, ExitStack() as ctx:
        cp = ctx.enter_context(tc.tile_pool(name="persist", bufs=1))
        pp = ctx.enter_context(tc.tile_pool(name="piecep", bufs=2))
        chp = ctx.enter_context(tc.tile_pool(name="chain", bufs=4))
        mlp_ps = ctx.enter_context(tc.tile_pool(name="mlpps", bufs=2, space="PSUM"))
        psb = CFG.get('psb', 2) if MAXC <= 4 else 1
        l3_ps = ctx.enter_context(tc.tile_pool(name="l3ps", bufs=psb, space="PSUM"))
        tp_ps = ctx.enter_context(tc.tile_pool(name="tpps", bufs=psb, space="PSUM"))

        cblob = cp.tile([128, CBYTES], mybir.dt.uint8, tag="cblob")
        lhs = cblob[:, 0:BLOB_B].bitcast(F16)
        consts = cblob[:, BLOB_B:BLOB_B + CONST_B].bitcast(F32)
        id32 = cblob[:, BLOB_B + CONST_B:CBYTES].bitcast(F32)

        # hoist the ACT table load under the input DMAs
        warm = cp.tile([128, 1], F32, tag="warm")
        nc.vector.memset(warm[:], 0.0)
        nc.scalar.activation(warm[:], warm[:], AF.Tanh, bias=0.0)

        def cv(col, parts=128):
            return consts[0:parts, col:col + 1]

        def eng(op, tail):
            e = CFG["eng_tail"].get(op) if tail else None
            e = e or CFG["eng"][op]
            return {"v": nc.vector, "p": nc.gpsimd, "a": nc.scalar}[e]

        # PE warm-up: dummy matmuls on a memset tile so the PE clock ramps
        # while the input DMAs are in flight.
        wm16 = cp.tile([128, 128], F16, tag="wm16")
        nc.vector.memset(wm16[:], 0.0)
        wmps = mlp_ps.tile([128, PSW], F32, tag="mlpps", name="wmps")
        for _w in range(CFG.get("warm_mms", 3)):
            nc.tensor.matmul(wmps[:, 0:128], wm16[0:64, 0:128],
                             wm16[0:64, 0:128], start=True, stop=True)

        def l1_mms(x16t, c0, C):
            """L1 matmuls for a piece; returns psum tile(s) for the tanh."""
            npc = 128 * C
            if CFG.get("comb_h", False) and 2 * npc <= PSW:
                ps = mlp_ps.tile([128, PSW], F32, tag="mlpps", name="l1ps")
                for h in (0, 1):
                    for so in range(0, npc, 512):
                        sw = min(512, npc - so)
                        nc.tensor.matmul(
                            ps[0:120, npc * h + so:npc * h + so + sw],
                            lhs[64 * h:64 * h + 64, 0:120],
                            x16t[64 * h:64 * h + 64,
                                 128 * c0 + so:128 * c0 + so + sw],
                            start=True, stop=True, tile_position=(64 * h, 0))
                return [ps]
            pss = []
            for h in (0, 1):
                ps = mlp_ps.tile([128, PSW], F32, tag="mlpps", name="l1ps")
                for so in range(0, npc, 512):
                    sw = min(512, npc - so)
                    nc.tensor.matmul(
                        ps[0:120, so:so + sw],
                        lhs[64 * h:64 * h + 64, 0:120],
                        x16t[64 * h:64 * h + 64, 128 * c0 + so:128 * c0 + so + sw],
                        start=True, stop=True, tile_position=(64 * h, 0))
                pss.append(ps)
            return pss

        def mlp_rest(l1ps, C):
            """L1 tanh, L2, L3 for a piece; returns corr [128, npc] f16."""
            npc = 128 * C
            h1sb = pp.tile([128, 2 * MN], F16, tag="h1", name="h1")
            h2sb = pp.tile([128, 2 * MN], F16, tag="h2", name="h2")
            if len(l1ps) == 1:      # both h halves in one psum tile
                nc.scalar.activation(
                    h1sb[0:120, 0:2 * npc],
                    l1ps[0][0:120, 0:2 * npc], AF.Tanh, bias=cv(C_BIAS1, 120))
                ps = mlp_ps.tile([128, PSW], F32, tag="mlpps", name="l2ps")
                for h in (0, 1):
                    for so in range(0, npc, 512):
                        sw = min(512, npc - so)
                        nc.tensor.matmul(
                            ps[0:120, npc * h + so:npc * h + so + sw],
                            lhs[0:120, 120:240],
                            h1sb[0:120, npc * h + so:npc * h + so + sw],
                            start=True, stop=True)
                nc.scalar.activation(
                    h2sb[0:120, 0:2 * npc],
                    ps[0:120, 0:2 * npc], AF.Tanh, bias=cv(C_BIAS2, 120))
            else:
                for h in (0, 1):
                    nc.scalar.activation(
                        h1sb[0:120, npc * h:npc * h + npc],
                        l1ps[h][0:120, 0:npc], AF.Tanh, bias=cv(C_BIAS1, 120))
                for h in (0, 1):
                    ps = mlp_ps.tile([128, PSW], F32, tag="mlpps", name="l2ps")
                    for so in range(0, npc, 512):
                        sw = min(512, npc - so)
                        nc.tensor.matmul(
                            ps[0:120, so:so + sw],
                            lhs[0:120, 120:240],
                            h1sb[0:120, npc * h + so:npc * h + so + sw],
                            start=True, stop=True)
                    nc.scalar.activation(
                        h2sb[0:120, npc * h:npc * h + npc],
                        ps[0:120, 0:npc], AF.Tanh, bias=cv(C_BIAS2, 120))
            ps3 = l3_ps.tile([128, MN], F32, tag="l3ps", name="l3ps")
            for h in (0, 1):
                for so in range(0, npc, 512):
                    sw = min(512, npc - so)
                    nc.tensor.matmul(
                        ps3[64 * h:64 * h + 64, so:so + sw],
                        lhs[0:120, 240:304],
                        h2sb[0:120, npc * h + so:npc * h + so + sw],
                        start=True, stop=True, tile_position=(0, 64 * h))
            corr = pp.tile([128, MN], F32, tag="corr", name="corr")
            nc.scalar.activation(corr[:, 0:npc], ps3[:, 0:npc],
                                 AF.Tanh, bias=cv(C_BIAS3))
            return corr

        def angle_part(xr32t, corr, CSg, gc0, GW, c0, C, tail):
            """Transpose + sin + combine for one piece, writing planes into its
            chain-group's CS tile ([t4 | ct | st] plane layout, GW chunks)."""
            npc = 128 * C
            tps = tp_ps.tile([128, MN], F32, tag="tpps", name="tps")
            for ci in range(C):
                nc.tensor.matmul(tps[:, 128 * ci:128 * ci + 128],
                                 xr32t[:, 128 * (c0 + ci):128 * (c0 + ci) + 128],
                                 id32[:], is_transpose=True, start=True, stop=False)
                nc.tensor.matmul(tps[:, 128 * ci:128 * ci + 128],
                                 corr[:, 128 * ci:128 * ci + 128],
                                 id32[:], is_transpose=True, start=False, stop=True)
            S24 = pp.tile([128, 2 * 96 * MAXC], F16, tag="S24", name="S24")
            in_v = tps[:, 0:npc].rearrange(
                "p (c h k g) -> p c h k g", c=C, h=2, k=8, g=8)[:, :, :, 0:6, :]
            s4v = S24[:, 0:96 * C].rearrange(
                "p (k c h g) -> p c h k g", k=6, c=C, h=2, g=8)
            s2v = S24[:, 96 * C:192 * C].rearrange(
                "p (k c h g) -> p c h k g", k=6, c=C, h=2, g=8)
            nc.scalar.activation(s2v, in_v, AF.Sin, bias=0.0, scale=0.5)
            nc.scalar.activation(s4v, in_v, AF.Sin, bias=0.0, scale=0.25)
            SQ = pp.tile([128, 2 * 96 * MAXC], F16, tag="SQ", name="SQ")
            eng("sq", tail).tensor_tensor(
                SQ[:, 0:192 * C], S24[:, 0:192 * C], S24[:, 0:192 * C], OP.mult)
            # [sq4 | sq2] -> [t4 | ct] planes of the group CS
            pc0 = c0 - gc0
            sq_v = SQ[:, 0:192 * C].rearrange(
                "p (x k c h g) -> p x k c h g", x=2, k=6, c=C, h=2, g=8)
            cs_v = CSg[:, 0:288 * GW].rearrange(
                "p (x k c h g) -> p x k c h g", x=3, k=6, c=GW, h=2, g=8)
            eng("ts", tail).tensor_scalar(
                cs_v[:, 0:2, :, pc0:pc0 + C], sq_v, -2.0, 1.0, OP.mult, OP.add)
            s2_v = S24[:, 96 * C:192 * C].rearrange(
                "p (k c h g) -> p k c h g", k=6, c=C, h=2, g=8)
            eng("stt", tail).scalar_tensor_tensor(
                cs_v[:, 2, :, pc0:pc0 + C], s2_v, 2.0,
                cs_v[:, 0, :, pc0:pc0 + C], OP.mult, OP.mult)

        def chain_part(CSg, pack, gc0, GW, tail):
            PL = 16 * GW
            m0 = 16 * gc0

            def ctj(j):
                return CSg[:, 96 * GW + 16 * GW * j:96 * GW + 16 * GW * j + PL]

            def stj(j):
                return CSg[:, 192 * GW + 16 * GW * j:192 * GW + 16 * GW * j + PL]

            CSv = CSg[:, 0:288 * GW].rearrange("p (x k m) -> p k x m",
                                               x=3, k=6, m=PL)

            def ch(tag, w=1):
                return chp.tile([128, w * 16 * NCH], F16, tag=tag, name=tag)

            def pv(t):
                return t[:, 0:2 * PL].rearrange("p (o x m) -> p o x m",
                                                o=1, x=2, m=PL)

            P = ch("P", 2)
            z = ch("z")
            u1 = ch("u1")
            eng("u1", tail).tensor_scalar(u1[:, 0:PL], ctj(5), sc["s5u1m"],
                                          sc["s5u1a"], OP.mult, OP.add)
            eng("x5", tail).scalar_tensor_tensor(P[:, 0:PL], stj(5), sc["s5xm"],
                                                 u1[:, 0:PL], OP.mult, OP.add)
            u3 = ch("u3")
            eng("u3", tail).tensor_scalar(u3[:, 0:PL], stj(5), sc["s5u3m"],
                                          sc["s5u3a"], OP.mult, OP.add)
            eng("y5", tail).scalar_tensor_tensor(P[:, PL:2 * PL], ctj(5),
                                                 sc["s5ym"], u3[:, 0:PL],
                                                 OP.mult, OP.add)
            u5 = ch("u5")
            eng("u5", tail).tensor_scalar(u5[:, 0:PL], stj(5), sc["s5u5m"],
                                          sc["s5u5a"], OP.mult, OP.add)
            eng("z5", tail).scalar_tensor_tensor(z[:, 0:PL], ctj(5), sc["s5zm"],
                                                 u5[:, 0:PL], OP.mult, OP.add)

            for j in (4, 3, 2, 1, 0):
                last = j == 0
                a_j, ca_j, sa_j = sc[f"a{j}"], sc[f"ca{j}"], sc[f"sa{j}"]
                dsa_j, cad_j = sc[f"dsa{j}"], sc[f"cad{j}"]
                q1 = ch("q1")
                e = eng("q1", tail)
                if e is nc.scalar:
                    e.activation(q1[:, 0:PL], z[:, 0:PL], AF.Identity,
                                 bias=cv(C_QB1 + j), scale=-sa_j)
                else:
                    e.tensor_scalar(q1[:, 0:PL], z[:, 0:PL], -sa_j, -dsa_j,
                                    OP.mult, OP.add)
                q2 = ch("q2")
                e = eng("q2", tail)
                if e is nc.scalar:
                    e.activation(q2[:, 0:PL], z[:, 0:PL], AF.Identity,
                                 bias=cv(C_QB2 + j), scale=ca_j)
                else:
                    e.tensor_scalar(q2[:, 0:PL], z[:, 0:PL], ca_j, cad_j,
                                    OP.mult, OP.add)
                TA = ch("TA", 2)
                pair = CSv[:, j:j + 1, 1:3, :]
                eng("ta", tail).tensor_tensor(pv(TA), pv(P), pair, OP.mult)
                Pn = pack if last else ch("P", 2)
                xn = Pn[:, m0:m0 + PL] if last else Pn[:, 0:PL]
                eng("xn", tail).scalar_tensor_tensor(
                    xn, TA[:, 0:PL], a_j, TA[:, PL:2 * PL], OP.add, OP.subtract)
                t3 = ch("t3")
                eng("t3", tail).tensor_tensor(t3[:, 0:PL], P[:, 0:PL], stj(j),
                                              OP.mult)
                t4 = ch("t4")
                eng("t4", tail).tensor_tensor(t4[:, 0:PL], P[:, PL:2 * PL],
                                              ctj(j), OP.mult)
                yr = ch("yr")
                eng("yr", tail).tensor_tensor(yr[:, 0:PL], t3[:, 0:PL],
                                              t4[:, 0:PL], OP.add)
                yn = Pn[:, 256 + m0:256 + m0 + PL] if last else Pn[:, PL:2 * PL]
                eng("yn", tail).scalar_tensor_tensor(
                    yn, yr[:, 0:PL], ca_j, q1[:, 0:PL], OP.mult, OP.add)
                zn = pack[:, 512 + m0:512 + m0 + PL] if last else ch("z")
                znv = zn if last else zn[:, 0:PL]
                eng("zn", tail).scalar_tensor_tensor(
                    znv, yr[:, 0:PL], sa_j, q2[:, 0:PL], OP.mult, OP.add)
                P, z = Pn, zn

        loop_ctx = tc.For_i(0, loop_n, 1) if loop_n else nullcontext()
        first = True
        with loop_ctx:
          for _rep in range(reps):
            x16t = cp.tile([128, 2048], F16, tag="x16", name="x16")
            xr32t = cp.tile([128, 2048], F32, tag="xr32", name="xr32")
            pack = cp.tile([128, 768], F16, tag="pack", name="pack")
            c0s = [sum(pieces[:i]) for i in range(len(pieces))]
            n1 = 128 * pieces[0]
            # piece-1 inputs first (they gate the whole pipeline), then the
            # rest; spread descriptor generation across idle engine queues
            deng = {"v": nc.vector, "a": nc.scalar, "s": nc.sync}
            nc.sync.dma_start(x16t[:, 0:n1], dx16.ap()[:, 0:n1])
            if first:
                deng[CFG.get("dma_cblob", "s")].dma_start(cblob[:], dcblob.ap())
                first = False
            deng[CFG.get("dma_xr1", "s")].dma_start(
                xr32t[:, 0:n1], dxr32.ap()[:, 0:n1])
            if n1 < 2048:
                nc.sync.dma_start(x16t[:, n1:2048], dx16.ap()[:, n1:2048])
                nc.sync.dma_start(xr32t[:, n1:2048], dxr32.ap()[:, n1:2048])
            groups = list(CFG["groups"])
            assert sum(groups) == len(pieces)
            group_idxs = []
            s = 0
            for g in groups:
                group_idxs.append(list(range(s, s + g)))
                s += g
            gl = {}          # piece idx -> (group tile, gc0, GW, is_last_piece)
            for gi, idxs in enumerate(group_idxs):
                GW = sum(pieces[i] for i in idxs)
                gc0 = c0s[idxs[0]]
                CSg = cp.tile([128, 288 * GW], F16, tag=f"CS{gi}", name=f"CS{gi}")
                for i in idxs:
                    gl[i] = (CSg, gc0, GW, gi, i == idxs[-1])
            l1ps = l1_mms(x16t, c0s[0], pieces[0])
            for pi, (c0, C) in enumerate(zip(c0s, pieces)):
                corr = mlp_rest(l1ps, C)
                if pi + 1 < len(pieces):
                    l1ps = l1_mms(x16t, c0s[pi + 1], pieces[pi + 1])
                CSg, gc0, GW, gi, last_in_group = gl[pi]
                tail = gi == len(group_idxs) - 1
                if CFG.get("hp_angle", True):
                    with tc.high_priority():
                        angle_part(xr32t, corr, CSg, gc0, GW, c0, C, tail)
                else:
                    angle_part(xr32t, corr, CSg, gc0, GW, c0, C, tail)
                if last_in_group:
                    if CFG.get("hp_angle", True):
                        with tc.high_priority():
                            chain_part(CSg, pack, gc0, GW, tail)
                    else:
                        chain_part(CSg, pack, gc0, GW, tail)
                    m0 = 16 * gc0
                    PL = 16 * GW
                    dv = dout.ap().rearrange("p (t m) -> p t m", t=3, m=256)
                    pvw = pack[:, 0:768].rearrange("p (t m) -> p t m", t=3, m=256)
                    if tail and CFG.get("dma3", False):
                        for t3i in range(3):
                            nc.sync.dma_start(dv[:, t3i:t3i + 1, m0:m0 + PL],
                                              pvw[:, t3i:t3i + 1, m0:m0 + PL])
                    else:
                        nc.sync.dma_start(dv[:, :, m0:m0 + PL],
                                          pvw[:, :, m0:m0 + PL])


_PROG_CACHE = {}


def _baked_scalars(inputs):
    fk = np.asarray(inputs["fk_params"], np.float32)
    alpha, a, d = fk[:, 0], fk[:, 1], fk[:, 2]
    ca, sa = np.cos(alpha), np.sin(alpha)
    t6 = np.array([a[6], -d[6] * sa[6], ca[6] * d[6]], np.float32)
    sc = {
        "s5u1m": a[6], "s5u1a": a[5], "s5xm": -t6[1],
        "s5u3m": ca[5] * a[6], "s5u3a": -sa[5] * t6[2] - d[5] * sa[5],
        "s5ym": ca[5] * t6[1],
        "s5u5m": sa[5] * a[6], "s5u5a": ca[5] * t6[2] + ca[5] * d[5],
        "s5zm": sa[5] * t6[1],
    }
    for j in range(5):
        sc[f"a{j}"] = a[j]
        sc[f"ca{j}"] = ca[j]
        sc[f"sa{j}"] = sa[j]
        sc[f"dsa{j}"] = d[j] * sa[j]
        sc[f"cad{j}"] = ca[j] * d[j]
    return {k: float(np.float32(v)) for k, v in sc.items()}


def _cfg_key():
    return (tuple(CFG["pieces"]), tuple(sorted(CFG["eng"].items())),
            tuple(sorted(CFG["eng_tail"].items())))


def _get_program(inputs, reps=1, loop_n=0):
    sc = _baked_scalars(inputs)
    key = (tuple(sorted(sc.items())), reps, loop_n, _cfg_key())
    if key in _PROG_CACHE:
        return _PROG_CACHE[key]
    nc = bacc.Bacc("TRN2", target_bir_lowering=False, debug=False,
                   enable_asserts=False)
    _emit_program(nc, sc, reps=reps, loop_n=loop_n)

    # Force Tanh and Sin to resolve to the one table set containing both
    # (silu_and_others), so the kernel pays a single ACT table load.
    import concourse.bacc as bacc_mod
    from concourse.hw_specs import get_activation_tables
    orig_fn = bacc_mod.get_activation_tables
    tabs = get_activation_tables(nc.m.arch)
    trig = {AF.Tanh, AF.Sin}
    doctored = {
        name: (set(funcs) if name == "silu_and_others" else set(funcs) - trig)
        for name, funcs in tabs.items()
    }
    bacc_mod.get_activation_tables = lambda arch: doctored
    try:
        nc.compile()
    finally:
        bacc_mod.get_activation_tables = orig_fn

    _PROG_CACHE[key] = nc
    return nc


LAST_RESULTS = None  # BassKernelResults of the most recent run (for test.py)


def _host_in_maps(inputs):
    imgs_x, imgs_r, cblob = _build_host_data(inputs)
    in_maps = []
    for core in range(N_CORES):
        in_maps.append({
            "x16": imgs_x[core],
            "xr32": imgs_r[core],
            "cblob": cblob,
        })
    return in_maps


def _jit_runner(nc):
    import jax
    from jax.sharding import Mesh, PartitionSpec, NamedSharding
    from jax.experimental.shard_map import shard_map
    from concourse import bass2jax
    bass2jax.install_neuronx_cc_hook()

    partition_name = nc.partition_id_tensor.name if nc.partition_id_tensor else None
    in_names, out_names, out_avals = [], [], []
    for alloc in nc.m.functions[0].allocations:
        if not isinstance(alloc, mybir.MemoryLocationSet):
            continue
        name = alloc.memorylocations[0].name
        if alloc.kind == "ExternalInput":
            if name != partition_name:
                in_names.append(name)
        elif alloc.kind == "ExternalOutput":
            out_names.append(name)
            out_avals.append(jax.core.ShapedArray(
                tuple(alloc.tensor_shape), mybir.dt.np(alloc.dtype)))
    all_in = in_names + out_names + ([partition_name] if partition_name else [])
    devices = jax.devices()[:N_CORES]
    mesh = Mesh(np.asarray(devices), ("core",))
    sh = NamedSharding(mesh, PartitionSpec("core"))

    def _body(*args):
        ops = list(args)
        if partition_name:
            ops.append(bass2jax.partition_id_tensor())
        outs = bass2jax._bass_exec_p.bind(
            *ops, out_avals=tuple(out_avals), in_names=tuple(all_in),
            out_names=tuple(out_names), lowering_input_output_aliases=(),
            sim_require_finite=True, sim_require_nnan=True, nc=nc)
        return tuple(outs)

    specs = (PartitionSpec("core"),) * (len(in_names) + len(out_names))
    ospec = (PartitionSpec("core"),) * len(out_names)
    f = jax.jit(shard_map(_body, mesh=mesh, in_specs=specs, out_specs=ospec,
                          check_rep=False))
    return f, in_names, out_avals, sh


def time_on_hw(inputs, n_lo=16, n_hi=256, iters=10):
    """Per-kernel HW time via an on-device For_i loop: slope of min wall
    between trip counts (includes ~2us loop back-edge per iteration)."""
    import time as _time
    import jax
    in_maps = _host_in_maps(inputs)
    mins = {}
    for loop_n in (n_lo, n_hi):
        nc = _get_program(inputs, loop_n=loop_n)
        f, in_names, out_avals, sh = _jit_runner(nc)
        cat = lambda n: np.concatenate(
            [np.asarray(in_maps[c][n]) for c in range(N_CORES)], axis=0)
        ci = [jax.device_put(cat(n), sh) for n in in_names]
        cz = [jax.device_put(
            np.zeros((N_CORES * a.shape[0], *a.shape[1:]), a.dtype), sh)
            for a in out_avals]
        jax.block_until_ready(f(*ci, *cz))
        best = float("inf")
        for _ in range(iters):
            t0 = _time.perf_counter()
            jax.block_until_ready(f(*ci, *cz))
            best = min(best, _time.perf_counter() - t0)
        mins[loop_n] = best
        print(f"[hw timing] loop_n={loop_n}: min wall {best*1e3:.2f} ms")
    slope_ns = (mins[n_hi] - mins[n_lo]) / (n_hi - n_lo) * 1e9
    print(f"[hw timing] -> {slope_ns:.0f} ns/kernel (incl ~2us loop overhead)")
    return slope_ns


def kernel(**inputs):
    global LAST_RESULTS
    j = np.asarray(inputs["joints"])
    assert j.shape == (B, 7), f"kernel hardcodes joints shape {(B, 7)}, got {j.shape}"
    nc = _get_program(inputs)
    in_maps = _host_in_maps(inputs)
    res = bass_utils.run_bass_kernel_spmd(nc, in_maps, core_ids=list(range(N_CORES)))
    LAST_RESULTS = res

    out = np.empty((B, 3), np.float32)
    for core in range(N_CORES):
        p = np.asarray(res.results[core]["out"]).astype(np.float32)  # [128, 768]
        # pack cols: [px(256) | py(256) | pz(256)], b_local = 256*p + m
        oc = p.reshape(128, 3, 256).transpose(0, 2, 1).reshape(BC, 3)
        out[core * BC:(core + 1) * BC] = oc
    return out
